# revision 7
# baseline (speedup 1.0000x reference)
"""Trainium2 Bass kernel for nn_DGLLayer_31482110279708 (moe_routing).

Strategy (data-parallel over batch n, 8 cores, 1 sample/core):
  - Host marshals per-core inputs: xT = x[:,n,:].T (f32), l_wT pre-split into
    bf16 hi/lo (for the big matmul), l_w & gw natural f32 (for the fp32 gate),
    plus bias/norm vectors.
  - Device per core:
      x_avg (mean over L) -> layernorm -> gate matvec1 (DVE, fp32 exact)
      -> gate matvec2 + relu (DVE, fp32) -> exact rank of logits
      (count of strictly-greater + equal-with-smaller-index; matches
      jax.lax.top_k ordering) -> top-k mask
      main matmul out[o,l] = sum_c l_w[o,c]*x[l,c] in bf16x2 (hi*hi + hi*lo +
      lo*hi, fp32 PSUM accum, ~1e-5 rel err) + l_b, then a second pass
      multiplies rows by the top-k mask.
  - Host inverts the rank permutation into idx and transposes out back to
    [L, N, O].
"""
import numpy as np

import concourse.bass as bass
import concourse.tile as tile
import concourse.mybir as mybir
from concourse import bacc, bass_utils
from concourse.bass_isa import ReduceOp
from concourse.masks import make_identity

L, NB, C, O = 1024, 8, 2048, 2048
TOPK = 1024
P = 128
CT = C // P          # 16 c-tiles
OT = O // P          # 16 o-tiles
LN_EPS = 1e-5
F32 = mybir.dt.float32
BF16 = mybir.dt.bfloat16
F16 = mybir.dt.float16
I16 = mybir.dt.int16
I32 = mybir.dt.int32
A = mybir.AluOpType


def build_nc():
    nc = bacc.Bacc("TRN2", target_bir_lowering=False, debug=False)

    xT_d = nc.dram_tensor("xT", [C, L], F32, kind="ExternalInput").ap()
    lwt_hi_d = nc.dram_tensor("lwt_hi", [C, O], BF16, kind="ExternalInput").ap()
    lwt_lo_d = nc.dram_tensor("lwt_lo", [C, O], BF16, kind="ExternalInput").ap()
    lw_d = nc.dram_tensor("lw", [O, C], F32, kind="ExternalInput").ap()
    gw_d = nc.dram_tensor("gw", [O, O], F32, kind="ExternalInput").ap()
    lb_d = nc.dram_tensor("lb", [O], F32, kind="ExternalInput").ap()
    gb_d = nc.dram_tensor("gb", [O], F32, kind="ExternalInput").ap()
    lng_d = nc.dram_tensor("lng", [C], F32, kind="ExternalInput").ap()
    lnb_d = nc.dram_tensor("lnb", [C], F32, kind="ExternalInput").ap()

    out_d = nc.dram_tensor("out_core", [O, L], F32, kind="ExternalOutput").ap()
    rank_d = nc.dram_tensor("rank_col", [P, OT], F32, kind="ExternalOutput").ap()
    logit_d = nc.dram_tensor("logits_col", [P, OT], F32, kind="ExternalOutput").ap()

    def col_ap(vec_d, n):
        # [n] DRAM vector -> [128, n/128] col-major SBUF load (elem i at
        # partition i%128, col i//128)
        return bass.AP(tensor=vec_d.tensor, offset=vec_d.offset,
                       ap=[[1, P], [P, n // P]])

    def row_ap(vec_d, n):
        # [n] DRAM vector viewed [n/128, 128] (row t = elems 128t..128t+127)
        return bass.AP(tensor=vec_d.tensor, offset=vec_d.offset,
                       ap=[[P, n // P], [1, P]])

    def bcast_ap(vec_d, n):
        # [n] DRAM vector broadcast across 128 partitions
        return bass.AP(tensor=vec_d.tensor, offset=vec_d.offset,
                       ap=[[0, P], [1, n]])

    with tile.TileContext(nc) as tc:
        with (
            tc.tile_pool(name="res", bufs=1) as res,
            tc.tile_pool(name="cols", bufs=1) as cols,
            tc.tile_pool(name="s32", bufs=3) as s32,
            tc.tile_pool(name="scr", bufs=2) as scr,
            tc.tile_pool(name="lwtp", bufs=2) as lwtp,
            tc.tile_pool(name="stg", bufs=3) as stg,
            tc.tile_pool(name="dram", bufs=1, space="DRAM") as dram,
            tc.tile_pool(name="psmm", bufs=4, space="PSUM") as psmm,
            tc.tile_pool(name="pstr", bufs=2, space="PSUM") as pstr,
        ):
            # ---- constants / small loads (sync queue) ----
            ident = cols.tile([P, P], F32, name="ident")
            make_identity(nc, ident)
            lb_col = cols.tile([P, OT], F32, name="lb_col")
            nc.gpsimd.dma_start(lb_col, col_ap(lb_d, O))
            gb_col = cols.tile([P, OT], F32, name="gb_col")
            nc.gpsimd.dma_start(gb_col, col_ap(gb_d, O))
            lng_col = cols.tile([P, CT], F32, name="lng_col")
            nc.gpsimd.dma_start(lng_col, col_ap(lng_d, C))
            lnb_col = cols.tile([P, CT], F32, name="lnb_col")
            nc.gpsimd.dma_start(lnb_col, col_ap(lnb_d, C))

            j_i16 = res.tile([P, O], I16, name="j_i16")
            nc.gpsimd.iota(j_i16, pattern=[[1, O]], base=0, channel_multiplier=0)
            j_f16 = res.tile([P, O], F16, name="j_f16")
            nc.vector.tensor_copy(j_f16, j_i16)
            idx_i32 = cols.tile([P, OT], I32, name="idx_i32")
            nc.gpsimd.iota(idx_i32, pattern=[[P, OT]], base=0, channel_multiplier=1)
            idx_f32 = cols.tile([P, OT], F32, name="idx_f32")
            nc.vector.tensor_copy(idx_f32, idx_i32)

            # ---- phase 1: x stream (gpsimd queue): x_avg + bf16 hi/lo split ----
            xt_hi = res.tile([P, CT, L], BF16, name="xt_hi")
            xt_lo = res.tile([P, CT, L], BF16, name="xt_lo")
            xsum = cols.tile([P, CT], F32, name="xsum")
            for kt in range(CT):
                xt32 = s32.tile([P, L], F32, name="xt32", tag="s32")
                nc.gpsimd.dma_start(xt32, xT_d[kt * P:(kt + 1) * P, :])
                nc.vector.tensor_reduce(xsum[:, kt:kt + 1], xt32,
                                        axis=mybir.AxisListType.X, op=A.add)
                nc.vector.tensor_copy(xt_hi[:, kt, :], xt32)
                nc.vector.tensor_tensor(out=xt_lo[:, kt, :], in0=xt32,
                                        in1=xt_hi[:, kt, :], op=A.subtract)

            # ---- phase 2: layernorm of x_avg (col layout [128, 16]) ----
            x_avg = cols.tile([P, CT], F32, name="x_avg")
            nc.vector.tensor_scalar_mul(x_avg, xsum, 1.0 / L)
            psum1 = cols.tile([P, 1], F32, name="psum1")
            nc.vector.tensor_reduce(psum1, x_avg, axis=mybir.AxisListType.X, op=A.add)
            nc.gpsimd.partition_all_reduce(psum1, psum1, P, ReduceOp.add)
            mu = cols.tile([P, 1], F32, name="mu")
            nc.vector.tensor_scalar_mul(mu, psum1, 1.0 / C)
            dcen = cols.tile([P, CT], F32, name="dcen")
            nc.vector.tensor_scalar(out=dcen, in0=x_avg, scalar1=mu, scalar2=None,
                                    op0=A.subtract)
            sq = cols.tile([P, CT], F32, name="sq")
            vsum = cols.tile([P, 1], F32, name="vsum")
            nc.vector.tensor_tensor_reduce(out=sq, in0=dcen, in1=dcen, scale=1.0,
                                           scalar=0.0, op0=A.mult, op1=A.add,
                                           accum_out=vsum)
            nc.gpsimd.partition_all_reduce(vsum, vsum, P, ReduceOp.add)
            var = cols.tile([P, 1], F32, name="var")
            nc.vector.tensor_scalar_mul(var, vsum, 1.0 / C)
            eps_t = cols.tile([P, 1], F32, name="eps_t")
            nc.vector.memset(eps_t, LN_EPS)
            std = cols.tile([P, 1], F32, name="std")
            nc.scalar.activation(std, var, mybir.ActivationFunctionType.Sqrt,
                                 bias=eps_t, scale=1.0)
            inv_std = cols.tile([P, 1], F32, name="inv_std")
            nc.vector.reciprocal(inv_std, std)
            xn_col = cols.tile([P, CT], F32, name="xn_col")
            nc.vector.tensor_scalar(out=xn_col, in0=dcen, scalar1=inv_std,
                                    scalar2=None, op0=A.mult)
            nc.vector.tensor_tensor(out=xn_col, in0=xn_col, in1=lng_col, op=A.mult)
            nc.vector.tensor_tensor(out=xn_col, in0=xn_col, in1=lnb_col, op=A.add)

            # ---- helper: col [128,16] -> DRAM row vector -> [128, n] bcast ----
            def col_to_bcast(col_t, n, tag):
                # transpose col layout [128, nt] -> [nt, 128] rows
                nt = n // P
                ps = pstr.tile([P, P], F32, name=f"ps_{tag}", tag="pstr")
                nc.tensor.transpose(ps[:nt, :], col_t, ident)
                row16 = cols.tile([CT, P], F32, name=f"row_{tag}")
                nc.vector.tensor_copy(row16[:nt, :], ps[:nt, :])
                drow = dram.tile([n], F32, name=f"drow_{tag}")
                nc.gpsimd.dma_start(row_ap(drow, n), row16[:nt, :])
                bc = res.tile([P, n], F32, name=f"bc_{tag}")
                nc.gpsimd.dma_start(bc, bcast_ap(drow, n))
                return bc

            xn_bc = col_to_bcast(xn_col, C, "xn")

            # ---- phase 3: gate matvec1 on DVE from natural l_w (fp32) ----
            gate_col = cols.tile([P, OT], F32, name="gate_col")
            for ot in range(OT):
                lw_t = s32.tile([P, C], F32, name="lw_t", tag="s32")
                nc.sync.dma_start(lw_t, lw_d[ot * P:(ot + 1) * P, :])
                so = scr.tile([P, C], F32, name="so", tag="scr")
                nc.vector.tensor_tensor_reduce(out=so, in0=lw_t, in1=xn_bc,
                                               scale=1.0, scalar=0.0, op0=A.mult,
                                               op1=A.add,
                                               accum_out=gate_col[:, ot:ot + 1])
            nc.vector.tensor_tensor(out=gate_col, in0=gate_col, in1=lb_col, op=A.add)

            gate_bc = col_to_bcast(gate_col, O, "gate")

            # ---- phase 4: gate matvec2 + relu on DVE from gw (fp32) ----
            logits_col = cols.tile([P, OT], F32, name="logits_col")
            for ot in range(OT):
                gw_t = s32.tile([P, O], F32, name="gw_t", tag="s32")
                nc.sync.dma_start(gw_t, gw_d[ot * P:(ot + 1) * P, :])
                so = scr.tile([P, O], F32, name="so2", tag="scr")
                nc.vector.tensor_tensor_reduce(out=so, in0=gw_t, in1=gate_bc,
                                               scale=1.0, scalar=0.0, op0=A.mult,
                                               op1=A.add,
                                               accum_out=logits_col[:, ot:ot + 1])
            nc.vector.tensor_tensor(out=logits_col, in0=logits_col, in1=gb_col,
                                    op=A.add)
            nc.vector.tensor_scalar_max(logits_col, logits_col, 0.0)
            nc.sync.dma_start(logit_d, logits_col)

            v_row = col_to_bcast(logits_col, O, "vrow")

            # ---- phase 5: exact rank (ties broken by smaller index first) ----
            gt_col = cols.tile([P, OT], F32, name="gt_col")
            eq_col = cols.tile([P, OT], F32, name="eq_col")
            for c in range(OT):
                sg = scr.tile([P, O], F32, name="sg", tag="scr")
                nc.vector.tensor_scalar(out=sg, in0=v_row,
                                        scalar1=logits_col[:, c:c + 1], scalar2=None,
                                        op0=A.is_gt, op1=A.add,
                                        accum_out=gt_col[:, c:c + 1])
                jlt = res.tile([P, O], F16, name="jlt", tag="jlt")
                nc.vector.tensor_scalar(out=jlt, in0=j_f16,
                                        scalar1=idx_f32[:, c:c + 1], scalar2=None,
                                        op0=A.is_lt)
                se = scr.tile([P, O], F32, name="se", tag="scr")
                nc.vector.scalar_tensor_tensor(out=se, in0=v_row,
                                               scalar=logits_col[:, c:c + 1],
                                               in1=jlt, op0=A.is_equal, op1=A.mult,
                                               accum_out=eq_col[:, c:c + 1])
            rank_col = cols.tile([P, OT], F32, name="rank_col_t")
            nc.vector.tensor_tensor(out=rank_col, in0=gt_col, in1=eq_col, op=A.add)
            nc.sync.dma_start(rank_d, rank_col)
            mask_col = cols.tile([P, OT], F32, name="mask_col")
            nc.vector.tensor_scalar(out=mask_col, in0=rank_col, scalar1=float(TOPK),
                                    scalar2=None, op0=A.is_lt)

            # ---- phase 6: main matmul (bf16x2, 3 terms) + bias, unmasked ----
            out_raw = dram.tile([O, L], F32, name="out_raw")
            for ot in range(OT):
                osl = slice(ot * P, (ot + 1) * P)
                wh = lwtp.tile([P, CT, P], BF16, name="wh", tag="wh")
                nc.gpsimd.dma_start(
                    wh, lwt_hi_d[:, osl].rearrange("(kt p) o -> p kt o", p=P))
                wl = lwtp.tile([P, CT, P], BF16, name="wl", tag="wl")
                nc.gpsimd.dma_start(
                    wl, lwt_lo_d[:, osl].rearrange("(kt p) o -> p kt o", p=P))
                pss = [psmm.tile([P, 512], F32, name=f"ps{lc}", tag="psmm")
                       for lc in range(2)]
                for kt in range(CT):
                    first = kt == 0
                    last = kt == CT - 1
                    for lc in range(2):
                        lsl = slice(lc * 512, (lc + 1) * 512)
                        nc.tensor.matmul(pss[lc], wh[:, kt, :], xt_hi[:, kt, lsl],
                                         start=first, stop=False)
                        nc.tensor.matmul(pss[lc], wh[:, kt, :], xt_lo[:, kt, lsl],
                                         start=False, stop=False)
                        nc.tensor.matmul(pss[lc], wl[:, kt, :], xt_hi[:, kt, lsl],
                                         start=False, stop=last)
                for lc in range(2):
                    st = stg.tile([P, 512], F32, name="st", tag="stg")
                    nc.vector.tensor_scalar(out=st, in0=pss[lc],
                                            scalar1=lb_col[:, ot:ot + 1],
                                            scalar2=None, op0=A.add)
                    nc.sync.dma_start(out_raw[osl, lc * 512:(lc + 1) * 512], st)

            # ---- phase 7: mask pass (zero non-top-k rows) ----
            for ot in range(OT):
                osl = slice(ot * P, (ot + 1) * P)
                rin = s32.tile([P, L], F32, name="rin", tag="s32")
                nc.sync.dma_start(rin, out_raw[osl, :])
                rout = s32.tile([P, L], F32, name="rout", tag="s32")
                nc.vector.tensor_scalar(out=rout, in0=rin,
                                        scalar1=mask_col[:, ot:ot + 1], scalar2=None,
                                        op0=A.mult)
                nc.sync.dma_start(out_d[osl, :], rout)

    nc.compile()
    return nc


_NC = None


def _get_nc():
    global _NC
    if _NC is None:
        _NC = build_nc()
    return _NC


def _prep_shared(l_w, gw, l_b, gb, ln_g, ln_b):
    import ml_dtypes
    lwt = np.ascontiguousarray(l_w.T.astype(np.float32))
    hi = lwt.astype(ml_dtypes.bfloat16)
    lo = (lwt - hi.astype(np.float32)).astype(ml_dtypes.bfloat16)
    return {
        "lwt_hi": hi,
        "lwt_lo": lo,
        "lw": np.ascontiguousarray(l_w.astype(np.float32)),
        "gw": np.ascontiguousarray(gw.astype(np.float32)),
        "lb": np.ascontiguousarray(l_b.astype(np.float32)),
        "gb": np.ascontiguousarray(gb.astype(np.float32)),
        "lng": np.ascontiguousarray(ln_g.astype(np.float32)),
        "lnb": np.ascontiguousarray(ln_b.astype(np.float32)),
    }


def kernel(x, l_w, l_b, ln_g, ln_b, gw, gb, top_k):
    x = np.asarray(x, dtype=np.float32)
    l_w = np.asarray(l_w, dtype=np.float32)
    l_b = np.asarray(l_b, dtype=np.float32)
    ln_g = np.asarray(ln_g, dtype=np.float32)
    ln_b = np.asarray(ln_b, dtype=np.float32)
    gw = np.asarray(gw, dtype=np.float32)
    gb = np.asarray(gb, dtype=np.float32)
    assert int(top_k) == TOPK and x.shape == (L, NB, C)

    nc = _get_nc()
    shared = _prep_shared(l_w, gw, l_b, gb, ln_g, ln_b)
    in_maps = []
    for n in range(NB):
        m = dict(shared)
        m["xT"] = np.ascontiguousarray(x[:, n, :].T)
        in_maps.append(m)

    # Sequential single-core dispatch: the 8-core shard_map path is unreliable
    # through this axon tunnel (large concat transfers); per-core calls reuse
    # one cached NEFF. Each sample still runs on its own NeuronCore program.
    results = None
    try:
        # Preferred: one SPMD launch across all 8 NeuronCores.
        res = bass_utils.run_bass_kernel_spmd(nc, in_maps, core_ids=list(range(NB)))
        results = res.results
    except Exception:
        results = None
    if results is None:
        try:
            # Fallback 1: sequential per-core launches.
            import jax
            devs = jax.devices()
            results = []
            for n in range(NB):
                with jax.default_device(devs[n % len(devs)]):
                    r1 = bass_utils.run_bass_kernel_spmd(nc, [in_maps[n]],
                                                         core_ids=[0])
                results.append(r1.results[0])
        except Exception:
            results = None
    if results is None:
        # Fallback 2: CoreSim (bit-accurate instruction simulator) — slow but
        # guarantees a correct result if the hardware path is unavailable.
        from concourse.bass_interp import CoreSim
        results = []
        for n in range(NB):
            sim = CoreSim(nc, require_finite=False, require_nnan=False)
            for k2, v2 in in_maps[n].items():
                sim.tensor(k2)[:] = v2
            sim.simulate(check_with_hw=False)
            results.append({
                "out_core": np.array(sim.tensor("out_core")),
                "rank_col": np.array(sim.tensor("rank_col")),
                "logits_col": np.array(sim.tensor("logits_col")),
            })

    out = np.empty((L, NB, O), dtype=np.float32)
    idx = np.empty((NB, TOPK), dtype=np.int32)
    for n in range(NB):
        r = results[n]
        out[:, n, :] = r["out_core"].T
        rank = r["rank_col"].T.reshape(-1).astype(np.int64)
        assert np.array_equal(np.sort(rank), np.arange(O)), "rank not a permutation"
        inv = np.empty(O, dtype=np.int32)
        inv[rank] = np.arange(O, dtype=np.int32)
        idx[n] = inv[:TOPK]
    return out, idx
